# revision 38
# baseline (speedup 1.0000x reference)
"""Trainium2 Bass kernel for the SLAYER-style 2-layer spiking encoder.

Pipeline per core (2 batches per core, 8 cores, pure data-parallel over batch):
  fc1 (PE, fp8-e4m3 DoubleRow, k-streamed)  ->  alpha-psp scans (DVE
  tensor_tensor_scan)  ->  membrane epilogue (ACT, c-major layout)  ->
  layer-1 spike chain (DVE, 4 ops/step, both batches in one 320-lane chain,
  20 time chunks x 25 steps + 16-step warmup)  ->  fc2 (PE, strided read of
  the step-major spike store)  ->  alpha-psp scans  ->  layer-2 spike chain
  (50 chunks x 10 + 10-step warmup)  ->  DMA out (host divides by the spike
  scale to recover 0/1 spikes).

Key algebraic facts exploited:
  * alpha_psp is linear and commutes with the feature-contracting matmuls:
    matmul the raw binary spikes (exactly representable in fp8/bf16), filter
    the (T,1024) result instead of (T,6300).
  * alpha_psp = two cascaded one-pole recurrences -> two tensor_tensor_scan
    instructions per tile:  p[t] = d*p[t-1] + v[t];  r[t] = d*r[t-1] + p[t];
    membrane drive  u[t] = c*d*r[t-1] - theta.
  * spike_dyn state decays by e^-1 per step, so time chunks processed in
    parallel SIMD lanes from zero state match the sequential result after a
    short warmup (residual 2e-7 .. 4e-3 vs. decision margins; layer-2 margin
    is ~9.0 so even large perturbations cannot flip the output).
  * spike stores hold V*s with V = -20 = bf16(d*cref'): exactly the value the
    refractory state update needs (Q += V*s), exactly representable, and the
    1/V is folded into the next scan scale / host rescale.  This lets the
    threshold op be a two-scalar tensor_scalar (4x DVE mode) and the membrane
    add a pure tensor_tensor (2x DVE mode); scalar_tensor_tensor (used for
    the two state decays) has no fast mode.

Chain-step recurrence in device variables (Zt = d*Z, Q = d*P of the
reference's scaled states):
    Zt = (Zt * d) + Q          # scalar_tensor_tensor
    M  = Zt + U_step           # tensor_tensor      (U is c-major: contiguous)
    S' = (M >= 0) * V          # tensor_scalar      (written to spike store)
    Q  = (Q * d) + S'          # scalar_tensor_tensor
"""

import os
import numpy as np
import ml_dtypes

DEBUG_DUMP = bool(os.environ.get("K_DEBUG"))   # also emit layer-1 spike store

# ---------------------------------------------------------------- constants
B_TOT = 16
B_PER = 2
N_CORES = 8
T = 500
F_IN = 6300
F_PAD = 6400
H1 = 1024
H2 = 20
KP1 = F_PAD // 256    # 25 fp8 DoubleRow k-pair tiles
OT1 = H1 // 128       # 8
KT2 = H1 // 128       # 8

THETA = 10.0
D = float(np.float32(np.exp(-1.0)))
C = float(np.float32(np.e))
CD = C * D
VSP = -20.0           # stored spike value = bf16-exact d*cref (cref'=-54.3662)

WARM1 = 8
NCH1, CHL1 = 25, 20
NSTEP1 = CHL1 + WARM1         # 28
LAN1 = B_PER * OT1 * NCH1     # 400 chain lanes per partition-row
WARM2 = 5
NCH2, CHL2 = 100, 5
NSTEP2 = CHL2 + WARM2         # 10
LAN2 = B_PER * NCH2           # 200

BF16 = ml_dtypes.bfloat16
E4M3 = ml_dtypes.float8_e4m3
_CACHE = {}


def _chunk_slices(i, chl, warm):
    """(first active chunk j0, in-chunk column c) for chain step i."""
    t0 = i - warm
    j0 = 0 if t0 >= 0 else (-t0 + chl - 1) // chl
    return j0, t0 + j0 * chl


def _build():
    import concourse.bass as bass
    import concourse.bacc as bacc
    import concourse.mybir as mybir
    import concourse.tile as tile

    f32 = mybir.dt.float32
    bf16 = mybir.dt.bfloat16
    fp8 = mybir.dt.float8e4
    MULT = mybir.AluOpType.mult
    ADD = mybir.AluOpType.add
    IS_GE = mybir.AluOpType.is_ge
    COPY = mybir.ActivationFunctionType.Copy
    DROW = mybir.MatmulPerfMode.DoubleRow

    nc = bacc.Bacc("TRN2", target_bir_lowering=False, debug=False,
                   num_devices=N_CORES)

    # x host-permuted to partition-major [b][p][kp][s][t]: SBUF-aligned DMA
    x_d = nc.dram_tensor("x", [B_PER, 128, KP1 * 2 * T], fp8,
                         kind="ExternalInput").ap()
    # host-permuted weights: [ot][kp][s][p][o] so one o-tile = one linear DMA
    w1t_d = nc.dram_tensor("w1t", [OT1, KP1 * 2 * 128 * 128], fp8,
                           kind="ExternalInput").ap()
    w2t_d = nc.dram_tensor("w2t", [128, KT2 * H2], bf16, kind="ExternalInput").ap()
    y_d = nc.dram_tensor("y", [H2, NSTEP2 * LAN2], bf16,
                         kind="ExternalOutput").ap()
    s1_d = (nc.dram_tensor("s1dump", [128, NSTEP1 * LAN1], bf16,
                           kind="ExternalOutput").ap() if DEBUG_DUMP else None)

    with tile.TileContext(nc) as tc:
        with (
            tc.tile_pool(name="xs", bufs=2) as xsp,
            tc.tile_pool(name="w1k", bufs=3) as w1kp,
            tc.tile_pool(name="wee", bufs=1) as wee,
            tc.tile_pool(name="ust", bufs=1) as ustp,
            tc.tile_pool(name="sst", bufs=1) as sstp,
            tc.tile_pool(name="scan", bufs=6) as scanp,
            tc.tile_pool(name="cst", bufs=1) as cstp,
            tc.tile_pool(name="state", bufs=3) as statep,
            tc.tile_pool(name="l2", bufs=1) as l2p,
            tc.tile_pool(name="ps", bufs=8, space="PSUM") as psp,
        ):
            dconst = cstp.tile([128, T], f32, tag="dconst")
            nc.gpsimd.memset(dconst[:], D)
            w2sb = wee.tile([128, KT2 * H2], bf16, tag="w2sb")
            nc.sync.dma_start(w2sb[:], w2t_d[:])

            # c-major membrane store: col = c*LAN1 + b*160 + g*20 + j,
            # holding U[t = j*CHL1 + c] = c*d*r[t-1] - theta  (bf16)
            u_cm = ustp.tile([128, CHL1 * LAN1], bf16, tag="ust")
            u5 = u_cm[:].rearrange("p (c b g j) -> p c b g j",
                                   c=CHL1, b=B_PER, g=OT1)
            nc.gpsimd.memset(u5[:, 0, :, :, 0], -THETA)   # t = 0
            # step-major spike store (contiguous per chain step)
            s_st = sstp.tile([128, NSTEP1 * LAN1], bf16, tag="sst")
            s5 = s_st[:].rearrange("p (i b g j) -> p i b g j",
                                   i=NSTEP1, b=B_PER, g=OT1)
            # layer-1 chain state (warmup phases run per batch, interleaved
            # with the other batch's fc1; main phase runs all 320 lanes)
            zt = statep.tile([128, LAN1], bf16, tag="state", name="z1")
            qt = statep.tile([128, LAN1], bf16, tag="state", name="q1")
            mt = statep.tile([128, LAN1], bf16, tag="state", name="m1")
            nc.gpsimd.memset(zt[:], 0.0)
            nc.gpsimd.memset(qt[:], 0.0)
            z5 = zt[:].rearrange("p (b g j) -> p b g j", b=B_PER, g=OT1)
            q5 = qt[:].rearrange("p (b g j) -> p b g j", b=B_PER, g=OT1)
            m5 = mt[:].rearrange("p (b g j) -> p b g j", b=B_PER, g=OT1)

            # ============== per-batch fc1 + scans + membrane epilogue
            # o-major: x resident per batch, one PSUM bank at a time, so each
            # o-tile's scans/epilogue pipeline inside fc1.
            # batch-0 x and the first weight tile first, so the PE can start
            # as soon as those land; batch-1 x prefetches during batch 0.
            xtiles = []
            for b in range(B_PER):
                xr = xsp.tile([128, KP1 * 2 * T], fp8, tag="xs", name=f"x_{b}")
                xtiles.append(xr[:].rearrange("p (kp s t) -> p kp s t",
                                              kp=KP1, s=2))
            xsrc = [x_d[b].rearrange("p (kp s t) -> p kp s t", kp=KP1, s=2)
                    for b in range(B_PER)]
            for j in range(0, KP1, 5):
                nc.sync.dma_start(xtiles[0][:, j:j + 5], xsrc[0][:, j:j + 5])
            for b in range(B_PER):
                x4 = xtiles[b]
                for ot in range(OT1):
                    w1o = w1kp.tile([128, KP1 * 2 * 128], fp8, tag="w1k",
                                    name=f"w1_{b}_{ot}")
                    nc.sync.dma_start(
                        w1o[:].rearrange("p (kp s o) -> p kp s o", kp=KP1, s=2),
                        w1t_d[ot].rearrange("(kp s p o) -> p kp s o",
                                            kp=KP1, s=2, p=128))
                    if b == 0 and 1 <= ot <= 5:
                        j = (ot - 1) * 5   # batch-1 x prefetch, deprioritized
                        nc.sync.dma_start(xtiles[1][:, j:j + 5],
                                          xsrc[1][:, j:j + 5])
                    w4 = w1o[:].rearrange("p (kp s o) -> p kp s o", kp=KP1, s=2)
                    v1 = psp.tile([128, T], f32, tag="ps", name=f"v1_{b}_{ot}")
                    for kp in range(KP1):
                        nc.tensor.matmul(
                            v1[:], w4[:, kp], x4[:, kp],
                            start=(kp == 0), stop=(kp == KP1 - 1),
                            perf_mode=DROW,
                        )
                    # alpha-psp scans (DVE) + c-major membrane writes (ACT)
                    p_t = scanp.tile([128, T], f32, tag="scan", name=f"p_{b}_{ot}")
                    r_t = scanp.tile([128, T], f32, tag="scan", name=f"r_{b}_{ot}")
                    nc.vector.tensor_tensor_scan(
                        p_t[:], dconst[:], v1[:], 0.0, op0=MULT, op1=ADD)
                    nc.vector.tensor_tensor_scan(
                        r_t[:], dconst[:], p_t[:], 0.0, op0=MULT, op1=ADD)
                    # chunk j=0, c>=1:  U[t=c] <- cd*r[c-1] - th
                    nc.scalar.activation(
                        u5[:, 1:, b, ot, 0], r_t[:, 0:CHL1 - 1],
                        COPY, bias=-THETA, scale=CD)
                    # chunks j>=1, all c: U[t=j*CHL1+c] <- cd*r[t-1] - th
                    out_ap = u5[:, :, b, ot, 1:].transpose([0, 2, 1])
                    in_ap = (r_t[:, CHL1 - 1:T - 1]
                             .rearrange("p (j c) -> p j c", j=NCH1 - 1))
                    nc.scalar.activation(out_ap, in_ap, COPY,
                                         bias=-THETA, scale=CD)

                # batch-b chain warmup: b=0's fills the DVE idle window while
                # the PE runs batch 1's fc1 (touches only this batch's lanes)
                bs = slice(b, b + 1)
                for i in range(WARM1):
                    _, ci = _chunk_slices(i, CHL1, WARM1)
                    zs, qs = z5[:, bs, :, 1:], q5[:, bs, :, 1:]
                    ms = m5[:, bs, :, 1:]
                    us = u5[:, ci, bs, :, 0:NCH1 - 1]
                    ss = s5[:, i, bs, :, 1:]
                    nc.vector.scalar_tensor_tensor(zs, zs, D, qs, op0=MULT, op1=ADD)
                    nc.vector.tensor_tensor(ms, zs, us, op=ADD)
                    nc.vector.tensor_scalar(ss, ms, 0.0, VSP, op0=IS_GE, op1=MULT)
                    nc.vector.scalar_tensor_tensor(qs, qs, D, ss, op0=MULT, op1=ADD)

            # ============== layer-1 spike chain main phase (400 lanes), with
            # fc2 matmuls pipelined in 4-step groups (the PE is idle here)
            FCG = 4
            v2t = [psp.tile([H2, T], f32, tag="ps", name=f"v2_{b}")
                   for b in range(B_PER)]
            for i in range(WARM1, NSTEP1):
                ci = i - WARM1
                zs, qs, ms = zt[:], qt[:], mt[:]
                us = u_cm[:, ci * LAN1:(ci + 1) * LAN1]
                ss = s_st[:, i * LAN1:(i + 1) * LAN1]
                nc.vector.scalar_tensor_tensor(zs, zs, D, qs, op0=MULT, op1=ADD)
                nc.vector.tensor_tensor(ms, zs, us, op=ADD)
                nc.vector.tensor_scalar(ss, ms, 0.0, VSP, op0=IS_GE, op1=MULT)
                nc.vector.scalar_tensor_tensor(qs, qs, D, ss, op0=MULT, op1=ADD)
                if ci % FCG == FCG - 1:
                    for b in range(B_PER):
                        v2r = v2t[b][:].rearrange("p (j c) -> p j c", j=NCH1)
                        for kt in range(KT2):
                            rhs = (s5[:, i - FCG + 1:i + 1, b, kt, :]
                                   .transpose([0, 2, 1]))
                            nc.tensor.matmul(
                                v2r[:, :, ci - FCG + 1:ci + 1],
                                w2sb[:, kt * H2:(kt + 1) * H2],
                                rhs,
                                start=(kt == 0), stop=(kt == KT2 - 1),
                            )

            # ============== layer 2
            # c-major membrane/spikes: col = c*LAN2 + b*NCH2 + j,
            # t = j*CHL2 + c
            u2 = l2p.tile([H2, CHL2 * LAN2], bf16, tag="u2")
            s2 = l2p.tile([H2, NSTEP2 * LAN2], bf16, tag="s2")
            u25 = u2[:].rearrange("p (c b j) -> p c b j", c=CHL2, b=B_PER)
            s25 = s2[:].rearrange("p (i b j) -> p i b j", i=NSTEP2, b=B_PER)
            nc.gpsimd.memset(u25[:, 0, :, 0], -THETA)
            for b in range(B_PER):
                v2 = v2t[b]
                p2t = scanp.tile([H2, T], f32, tag="scan", name=f"p2_{b}")
                r2t = scanp.tile([H2, T], f32, tag="scan", name=f"r2_{b}")
                nc.vector.tensor_tensor_scan(
                    p2t[:], dconst[0:H2, :], v2[:], 0.0, op0=MULT, op1=ADD)
                nc.vector.tensor_tensor_scan(
                    r2t[:], dconst[0:H2, :], p2t[:], 0.0, op0=MULT, op1=ADD)
                nc.scalar.activation(
                    u25[:, 1:, b, 0], r2t[:, 0:CHL2 - 1],
                    COPY, bias=-THETA, scale=CD / VSP)
                out_ap = u25[:, :, b, 1:].transpose([0, 2, 1])
                in_ap = (r2t[:, CHL2 - 1:T - 1]
                         .rearrange("p (j c) -> p j c", j=NCH2 - 1))
                nc.scalar.activation(out_ap, in_ap, COPY,
                                     bias=-THETA, scale=CD / VSP)

            # layer-2 spike chain; store keeps V*s (host divides by V)
            z2 = l2p.tile([H2, LAN2], bf16, tag="z2")
            q2 = l2p.tile([H2, LAN2], bf16, tag="q2")
            m2 = l2p.tile([H2, LAN2], bf16, tag="m2")
            nc.gpsimd.memset(z2[:], 0.0)
            nc.gpsimd.memset(q2[:], 0.0)
            z2v = z2[:].rearrange("p (b j) -> p b j", b=B_PER)
            q2v = q2[:].rearrange("p (b j) -> p b j", b=B_PER)
            m2v = m2[:].rearrange("p (b j) -> p b j", b=B_PER)
            for i in range(NSTEP2):
                j0, ci = _chunk_slices(i, CHL2, WARM2)
                if j0 == 0:
                    zs, qs, ms = z2[:], q2[:], m2[:]
                    us = u2[:, ci * LAN2:(ci + 1) * LAN2]
                    ss = s2[:, i * LAN2:(i + 1) * LAN2]
                else:
                    zs, qs = z2v[:, :, j0:], q2v[:, :, j0:]
                    ms = m2v[:, :, j0:]
                    # state slot j warms up on chunk j-j0's history
                    us = u25[:, ci, :, 0:NCH2 - j0]
                    ss = s25[:, i, :, j0:]
                nc.vector.scalar_tensor_tensor(zs, zs, D, qs, op0=MULT, op1=ADD)
                nc.vector.tensor_tensor(ms, zs, us, op=ADD)
                nc.vector.tensor_scalar(ss, ms, 0.0, VSP, op0=IS_GE, op1=MULT)
                nc.vector.scalar_tensor_tensor(qs, qs, D, ss, op0=MULT, op1=ADD)

            # ship the whole step-major spike store; host extracts output phase
            nc.sync.dma_start(y_d[:], s2[:])
            if DEBUG_DUMP:
                nc.sync.dma_start(s1_d[:], s_st[:])

    nc.compile()
    return nc


def _get_nc():
    if "nc" not in _CACHE:
        _CACHE["nc"] = _build()
    return _CACHE["nc"]


def _prep_inputs(downsampled, w1, w2):
    x = np.ascontiguousarray(downsampled.reshape(B_TOT, F_IN, T))
    xpad = np.zeros((B_TOT, F_PAD, T), dtype=E4M3)
    xpad[:, :F_IN] = x.astype(E4M3)          # binary spikes: exact in e4m3
    # [b, f, t] -> [b][p][kp][s][t] (partition-major, SBUF-aligned linear DMA)
    xpad = np.ascontiguousarray(
        xpad.reshape(B_TOT, KP1, 2, 128, T).transpose(0, 3, 1, 2, 4)
        .reshape(B_TOT, 128, KP1 * 2 * T))
    w1t = np.zeros((F_PAD, H1), dtype=E4M3)
    w1t[:F_IN] = np.ascontiguousarray(w1.T).astype(E4M3)
    # [f, o] -> [ot][kp][s][p][o_local] so one o-tile is a linear DMA
    w1t = np.ascontiguousarray(
        w1t.reshape(KP1, 2, 128, OT1, 128).transpose(3, 0, 1, 2, 4)
        .reshape(OT1, KP1 * 2 * 128 * 128))
    w2t = np.ascontiguousarray(
        w2.T.reshape(KT2, 128, H2).transpose(1, 0, 2).reshape(128, KT2 * H2)
    ).astype(BF16)
    return [
        {"x": np.ascontiguousarray(xpad[c * B_PER:(c + 1) * B_PER]),
         "w1t": w1t, "w2t": w2t}
        for c in range(N_CORES)
    ]


def kernel(downsampled: np.ndarray, w1: np.ndarray, w2: np.ndarray) -> np.ndarray:
    from concourse.bass_utils import run_bass_kernel_spmd

    nc = _get_nc()
    in_maps = _prep_inputs(downsampled, w1, w2)
    res = run_bass_kernel_spmd(nc, in_maps, core_ids=list(range(N_CORES)))
    out = np.stack([res.results[c]["y"] for c in range(N_CORES)])
    # y is the step-major spike store: [o2, (i b j)]; output phase i>=WARM2
    # holds spikes for t = j*CHL2 + (i - WARM2), scaled by V.
    out = out.reshape(N_CORES, H2, NSTEP2, B_PER, NCH2).astype(np.float32)
    out = out[:, :, WARM2:]                      # (core, o2, c, b, j)
    out = out.transpose(0, 3, 1, 4, 2)           # core, b, o2, j, c
    out = out.reshape(B_TOT, H2, T) / np.float32(VSP)   # V*s -> s (exact)
    return np.ascontiguousarray(out.astype(np.float32))


# revision 39
# speedup vs baseline: 1.0188x; 1.0188x over previous
"""Trainium2 Bass kernel for the SLAYER-style 2-layer spiking encoder.

Pipeline per core (2 batches per core, 8 cores, pure data-parallel over batch):
  fc1 (PE, fp8-e4m3 DoubleRow, k-streamed)  ->  alpha-psp scans (DVE
  tensor_tensor_scan)  ->  membrane epilogue (ACT, c-major layout)  ->
  layer-1 spike chain (DVE, 4 ops/step, both batches in one 320-lane chain,
  20 time chunks x 25 steps + 16-step warmup)  ->  fc2 (PE, strided read of
  the step-major spike store)  ->  alpha-psp scans  ->  layer-2 spike chain
  (50 chunks x 10 + 10-step warmup)  ->  DMA out (host divides by the spike
  scale to recover 0/1 spikes).

Key algebraic facts exploited:
  * alpha_psp is linear and commutes with the feature-contracting matmuls:
    matmul the raw binary spikes (exactly representable in fp8/bf16), filter
    the (T,1024) result instead of (T,6300).
  * alpha_psp = two cascaded one-pole recurrences -> two tensor_tensor_scan
    instructions per tile:  p[t] = d*p[t-1] + v[t];  r[t] = d*r[t-1] + p[t];
    membrane drive  u[t] = c*d*r[t-1] - theta.
  * spike_dyn state decays by e^-1 per step, so time chunks processed in
    parallel SIMD lanes from zero state match the sequential result after a
    short warmup (residual 2e-7 .. 4e-3 vs. decision margins; layer-2 margin
    is ~9.0 so even large perturbations cannot flip the output).
  * spike stores hold V*s with V = -20 = bf16(d*cref'): exactly the value the
    refractory state update needs (Q += V*s), exactly representable, and the
    1/V is folded into the next scan scale / host rescale.  This lets the
    threshold op be a two-scalar tensor_scalar (4x DVE mode) and the membrane
    add a pure tensor_tensor (2x DVE mode); scalar_tensor_tensor (used for
    the two state decays) has no fast mode.

Chain-step recurrence in device variables (Zt = d*Z, Q = d*P of the
reference's scaled states):
    Zt = (Zt * d) + Q          # scalar_tensor_tensor
    M  = Zt + U_step           # tensor_tensor      (U is c-major: contiguous)
    S' = (M >= 0) * V          # tensor_scalar      (written to spike store)
    Q  = (Q * d) + S'          # scalar_tensor_tensor
"""

import os
import numpy as np
import ml_dtypes

DEBUG_DUMP = bool(os.environ.get("K_DEBUG"))   # also emit layer-1 spike store

# ---------------------------------------------------------------- constants
B_TOT = 16
B_PER = 2
N_CORES = 8
T = 500
F_IN = 6300
F_PAD = 6400
H1 = 1024
H2 = 20
KP1 = F_PAD // 256    # 25 fp8 DoubleRow k-pair tiles
OT1 = H1 // 128       # 8
KT2 = H1 // 128       # 8

THETA = 10.0
D = float(np.float32(np.exp(-1.0)))
C = float(np.float32(np.e))
CD = C * D
VSP = -20.0           # stored spike value = bf16-exact d*cref (cref'=-54.3662)

WARM1 = 8
NCH1, CHL1 = 25, 20
NSTEP1 = CHL1 + WARM1         # 28
LAN1 = B_PER * OT1 * NCH1     # 400 chain lanes per partition-row
WARM2 = 5
NCH2, CHL2 = 100, 5
NSTEP2 = CHL2 + WARM2         # 10
LAN2 = B_PER * NCH2           # 200

BF16 = ml_dtypes.bfloat16
E4M3 = ml_dtypes.float8_e4m3
_CACHE = {}


def _chunk_slices(i, chl, warm):
    """(first active chunk j0, in-chunk column c) for chain step i."""
    t0 = i - warm
    j0 = 0 if t0 >= 0 else (-t0 + chl - 1) // chl
    return j0, t0 + j0 * chl


def _build():
    import concourse.bass as bass
    import concourse.bacc as bacc
    import concourse.mybir as mybir
    import concourse.tile as tile

    f32 = mybir.dt.float32
    bf16 = mybir.dt.bfloat16
    fp8 = mybir.dt.float8e4
    MULT = mybir.AluOpType.mult
    ADD = mybir.AluOpType.add
    IS_GE = mybir.AluOpType.is_ge
    COPY = mybir.ActivationFunctionType.Copy
    DROW = mybir.MatmulPerfMode.DoubleRow

    nc = bacc.Bacc("TRN2", target_bir_lowering=False, debug=False,
                   num_devices=N_CORES)

    # x host-permuted to partition-major [b][p][kp][s][t]: SBUF-aligned DMA
    x_d = nc.dram_tensor("x", [B_PER, 128, KP1 * 2 * T], fp8,
                         kind="ExternalInput").ap()
    # host-permuted weights: [ot][kp][s][p][o] so one o-tile = one linear DMA
    w1t_d = nc.dram_tensor("w1t", [OT1, KP1 * 2 * 128 * 128], fp8,
                           kind="ExternalInput").ap()
    w2t_d = nc.dram_tensor("w2t", [128, KT2 * H2], bf16, kind="ExternalInput").ap()
    y_d = nc.dram_tensor("y", [H2, NSTEP2 * LAN2], bf16,
                         kind="ExternalOutput").ap()
    s1_d = (nc.dram_tensor("s1dump", [128, NSTEP1 * LAN1], bf16,
                           kind="ExternalOutput").ap() if DEBUG_DUMP else None)

    with tile.TileContext(nc) as tc:
        with (
            tc.tile_pool(name="xs", bufs=2) as xsp,
            tc.tile_pool(name="w1k", bufs=3) as w1kp,
            tc.tile_pool(name="wee", bufs=1) as wee,
            tc.tile_pool(name="ust", bufs=1) as ustp,
            tc.tile_pool(name="sst", bufs=1) as sstp,
            tc.tile_pool(name="scan", bufs=6) as scanp,
            tc.tile_pool(name="cst", bufs=1) as cstp,
            tc.tile_pool(name="state", bufs=3) as statep,
            tc.tile_pool(name="l2", bufs=1) as l2p,
            tc.tile_pool(name="ps", bufs=8, space="PSUM") as psp,
        ):
            dconst = cstp.tile([128, T], f32, tag="dconst")
            nc.gpsimd.memset(dconst[:], D)
            w2sb = wee.tile([128, KT2 * H2], bf16, tag="w2sb")
            nc.sync.dma_start(w2sb[:], w2t_d[:])

            # c-major membrane store: col = c*LAN1 + b*160 + g*20 + j,
            # holding U[t = j*CHL1 + c] = c*d*r[t-1] - theta  (bf16)
            u_cm = ustp.tile([128, CHL1 * LAN1], bf16, tag="ust")
            u5 = u_cm[:].rearrange("p (c b g j) -> p c b g j",
                                   c=CHL1, b=B_PER, g=OT1)
            nc.gpsimd.memset(u5[:, 0, :, :, 0], -THETA)   # t = 0
            # step-major spike store (contiguous per chain step)
            s_st = sstp.tile([128, NSTEP1 * LAN1], bf16, tag="sst")
            s5 = s_st[:].rearrange("p (i b g j) -> p i b g j",
                                   i=NSTEP1, b=B_PER, g=OT1)
            # layer-1 chain state (warmup phases run per batch, interleaved
            # with the other batch's fc1; main phase runs all 320 lanes)
            zt = statep.tile([128, LAN1], bf16, tag="state", name="z1")
            qt = statep.tile([128, LAN1], bf16, tag="state", name="q1")
            mt = statep.tile([128, LAN1], bf16, tag="state", name="m1")
            nc.gpsimd.memset(zt[:], 0.0)
            nc.gpsimd.memset(qt[:], 0.0)
            z5 = zt[:].rearrange("p (b g j) -> p b g j", b=B_PER, g=OT1)
            q5 = qt[:].rearrange("p (b g j) -> p b g j", b=B_PER, g=OT1)
            m5 = mt[:].rearrange("p (b g j) -> p b g j", b=B_PER, g=OT1)

            # ============== per-batch fc1 + scans + membrane epilogue
            # o-major: x resident per batch, one PSUM bank at a time, so each
            # o-tile's scans/epilogue pipeline inside fc1.
            # batch-0 x and the first weight tile first, so the PE can start
            # as soon as those land; batch-1 x prefetches during batch 0.
            xtiles = []
            for b in range(B_PER):
                xr = xsp.tile([128, KP1 * 2 * T], fp8, tag="xs", name=f"x_{b}")
                xtiles.append(xr[:].rearrange("p (kp s t) -> p kp s t",
                                              kp=KP1, s=2))
            xsrc = [x_d[b].rearrange("p (kp s t) -> p kp s t", kp=KP1, s=2)
                    for b in range(B_PER)]
            for j in range(0, KP1, 5):
                nc.sync.dma_start(xtiles[0][:, j:j + 5], xsrc[0][:, j:j + 5])
            for b in range(B_PER):
                x4 = xtiles[b]
                for ot in range(OT1):
                    w1o = w1kp.tile([128, KP1 * 2 * 128], fp8, tag="w1k",
                                    name=f"w1_{b}_{ot}")
                    wv = w1o[:].rearrange("p (kp s o) -> p kp s o", kp=KP1, s=2)
                    wsc = w1t_d[ot].rearrange("(kp s p o) -> p kp s o",
                                              kp=KP1, s=2, p=128)
                    nc.sync.dma_start(wv[:, 0:13], wsc[:, 0:13])
                    nc.sync.dma_start(wv[:, 13:KP1], wsc[:, 13:KP1])
                    if b == 0 and 1 <= ot <= 5:
                        j = (ot - 1) * 5   # batch-1 x prefetch, deprioritized
                        nc.sync.dma_start(xtiles[1][:, j:j + 5],
                                          xsrc[1][:, j:j + 5])
                    w4 = w1o[:].rearrange("p (kp s o) -> p kp s o", kp=KP1, s=2)
                    v1 = psp.tile([128, T], f32, tag="ps", name=f"v1_{b}_{ot}")
                    for kp in range(KP1):
                        nc.tensor.matmul(
                            v1[:], w4[:, kp], x4[:, kp],
                            start=(kp == 0), stop=(kp == KP1 - 1),
                            perf_mode=DROW,
                        )
                    # alpha-psp scans (DVE) + c-major membrane writes (ACT)
                    p_t = scanp.tile([128, T], f32, tag="scan", name=f"p_{b}_{ot}")
                    r_t = scanp.tile([128, T], f32, tag="scan", name=f"r_{b}_{ot}")
                    nc.vector.tensor_tensor_scan(
                        p_t[:], dconst[:], v1[:], 0.0, op0=MULT, op1=ADD)
                    nc.vector.tensor_tensor_scan(
                        r_t[:], dconst[:], p_t[:], 0.0, op0=MULT, op1=ADD)
                    # chunk j=0, c>=1:  U[t=c] <- cd*r[c-1] - th
                    nc.scalar.activation(
                        u5[:, 1:, b, ot, 0], r_t[:, 0:CHL1 - 1],
                        COPY, bias=-THETA, scale=CD)
                    # chunks j>=1, all c: U[t=j*CHL1+c] <- cd*r[t-1] - th
                    out_ap = u5[:, :, b, ot, 1:].transpose([0, 2, 1])
                    in_ap = (r_t[:, CHL1 - 1:T - 1]
                             .rearrange("p (j c) -> p j c", j=NCH1 - 1))
                    nc.scalar.activation(out_ap, in_ap, COPY,
                                         bias=-THETA, scale=CD)

                # batch-b chain warmup: b=0's fills the DVE idle window while
                # the PE runs batch 1's fc1 (touches only this batch's lanes)
                bs = slice(b, b + 1)
                for i in range(WARM1):
                    _, ci = _chunk_slices(i, CHL1, WARM1)
                    zs, qs = z5[:, bs, :, 1:], q5[:, bs, :, 1:]
                    ms = m5[:, bs, :, 1:]
                    us = u5[:, ci, bs, :, 0:NCH1 - 1]
                    ss = s5[:, i, bs, :, 1:]
                    nc.vector.scalar_tensor_tensor(zs, zs, D, qs, op0=MULT, op1=ADD)
                    nc.vector.tensor_tensor(ms, zs, us, op=ADD)
                    nc.vector.tensor_scalar(ss, ms, 0.0, VSP, op0=IS_GE, op1=MULT)
                    nc.vector.scalar_tensor_tensor(qs, qs, D, ss, op0=MULT, op1=ADD)

            # ============== layer-1 spike chain main phase (400 lanes), with
            # fc2 matmuls pipelined in 4-step groups (the PE is idle here)
            FCG = 4
            v2t = [psp.tile([H2, T], f32, tag="ps", name=f"v2_{b}")
                   for b in range(B_PER)]
            for i in range(WARM1, NSTEP1):
                ci = i - WARM1
                zs, qs, ms = zt[:], qt[:], mt[:]
                us = u_cm[:, ci * LAN1:(ci + 1) * LAN1]
                ss = s_st[:, i * LAN1:(i + 1) * LAN1]
                nc.vector.scalar_tensor_tensor(zs, zs, D, qs, op0=MULT, op1=ADD)
                nc.vector.tensor_tensor(ms, zs, us, op=ADD)
                nc.vector.tensor_scalar(ss, ms, 0.0, VSP, op0=IS_GE, op1=MULT)
                nc.vector.scalar_tensor_tensor(qs, qs, D, ss, op0=MULT, op1=ADD)
                if ci % FCG == FCG - 1:
                    for b in range(B_PER):
                        v2r = v2t[b][:].rearrange("p (j c) -> p j c", j=NCH1)
                        for kt in range(KT2):
                            rhs = (s5[:, i - FCG + 1:i + 1, b, kt, :]
                                   .transpose([0, 2, 1]))
                            nc.tensor.matmul(
                                v2r[:, :, ci - FCG + 1:ci + 1],
                                w2sb[:, kt * H2:(kt + 1) * H2],
                                rhs,
                                start=(kt == 0), stop=(kt == KT2 - 1),
                            )

            # ============== layer 2
            # c-major membrane/spikes: col = c*LAN2 + b*NCH2 + j,
            # t = j*CHL2 + c
            u2 = l2p.tile([H2, CHL2 * LAN2], bf16, tag="u2")
            s2 = l2p.tile([H2, NSTEP2 * LAN2], bf16, tag="s2")
            u25 = u2[:].rearrange("p (c b j) -> p c b j", c=CHL2, b=B_PER)
            s25 = s2[:].rearrange("p (i b j) -> p i b j", i=NSTEP2, b=B_PER)
            nc.gpsimd.memset(u25[:, 0, :, 0], -THETA)
            for b in range(B_PER):
                v2 = v2t[b]
                p2t = scanp.tile([H2, T], f32, tag="scan", name=f"p2_{b}")
                r2t = scanp.tile([H2, T], f32, tag="scan", name=f"r2_{b}")
                nc.vector.tensor_tensor_scan(
                    p2t[:], dconst[0:H2, :], v2[:], 0.0, op0=MULT, op1=ADD)
                nc.vector.tensor_tensor_scan(
                    r2t[:], dconst[0:H2, :], p2t[:], 0.0, op0=MULT, op1=ADD)
                nc.scalar.activation(
                    u25[:, 1:, b, 0], r2t[:, 0:CHL2 - 1],
                    COPY, bias=-THETA, scale=CD / VSP)
                out_ap = u25[:, :, b, 1:].transpose([0, 2, 1])
                in_ap = (r2t[:, CHL2 - 1:T - 1]
                         .rearrange("p (j c) -> p j c", j=NCH2 - 1))
                nc.scalar.activation(out_ap, in_ap, COPY,
                                     bias=-THETA, scale=CD / VSP)

            # layer-2 spike chain; store keeps V*s (host divides by V)
            z2 = l2p.tile([H2, LAN2], bf16, tag="z2")
            q2 = l2p.tile([H2, LAN2], bf16, tag="q2")
            m2 = l2p.tile([H2, LAN2], bf16, tag="m2")
            nc.gpsimd.memset(z2[:], 0.0)
            nc.gpsimd.memset(q2[:], 0.0)
            z2v = z2[:].rearrange("p (b j) -> p b j", b=B_PER)
            q2v = q2[:].rearrange("p (b j) -> p b j", b=B_PER)
            m2v = m2[:].rearrange("p (b j) -> p b j", b=B_PER)
            for i in range(NSTEP2):
                j0, ci = _chunk_slices(i, CHL2, WARM2)
                if j0 == 0:
                    zs, qs, ms = z2[:], q2[:], m2[:]
                    us = u2[:, ci * LAN2:(ci + 1) * LAN2]
                    ss = s2[:, i * LAN2:(i + 1) * LAN2]
                else:
                    zs, qs = z2v[:, :, j0:], q2v[:, :, j0:]
                    ms = m2v[:, :, j0:]
                    # state slot j warms up on chunk j-j0's history
                    us = u25[:, ci, :, 0:NCH2 - j0]
                    ss = s25[:, i, :, j0:]
                nc.vector.scalar_tensor_tensor(zs, zs, D, qs, op0=MULT, op1=ADD)
                nc.vector.tensor_tensor(ms, zs, us, op=ADD)
                nc.vector.tensor_scalar(ss, ms, 0.0, VSP, op0=IS_GE, op1=MULT)
                nc.vector.scalar_tensor_tensor(qs, qs, D, ss, op0=MULT, op1=ADD)

            # ship the whole step-major spike store; host extracts output phase
            nc.sync.dma_start(y_d[:], s2[:])
            if DEBUG_DUMP:
                nc.sync.dma_start(s1_d[:], s_st[:])

    nc.compile()
    return nc


def _get_nc():
    if "nc" not in _CACHE:
        _CACHE["nc"] = _build()
    return _CACHE["nc"]


def _prep_inputs(downsampled, w1, w2):
    x = np.ascontiguousarray(downsampled.reshape(B_TOT, F_IN, T))
    xpad = np.zeros((B_TOT, F_PAD, T), dtype=E4M3)
    xpad[:, :F_IN] = x.astype(E4M3)          # binary spikes: exact in e4m3
    # [b, f, t] -> [b][p][kp][s][t] (partition-major, SBUF-aligned linear DMA)
    xpad = np.ascontiguousarray(
        xpad.reshape(B_TOT, KP1, 2, 128, T).transpose(0, 3, 1, 2, 4)
        .reshape(B_TOT, 128, KP1 * 2 * T))
    w1t = np.zeros((F_PAD, H1), dtype=E4M3)
    w1t[:F_IN] = np.ascontiguousarray(w1.T).astype(E4M3)
    # [f, o] -> [ot][kp][s][p][o_local] so one o-tile is a linear DMA
    w1t = np.ascontiguousarray(
        w1t.reshape(KP1, 2, 128, OT1, 128).transpose(3, 0, 1, 2, 4)
        .reshape(OT1, KP1 * 2 * 128 * 128))
    w2t = np.ascontiguousarray(
        w2.T.reshape(KT2, 128, H2).transpose(1, 0, 2).reshape(128, KT2 * H2)
    ).astype(BF16)
    return [
        {"x": np.ascontiguousarray(xpad[c * B_PER:(c + 1) * B_PER]),
         "w1t": w1t, "w2t": w2t}
        for c in range(N_CORES)
    ]


def kernel(downsampled: np.ndarray, w1: np.ndarray, w2: np.ndarray) -> np.ndarray:
    from concourse.bass_utils import run_bass_kernel_spmd

    nc = _get_nc()
    in_maps = _prep_inputs(downsampled, w1, w2)
    res = run_bass_kernel_spmd(nc, in_maps, core_ids=list(range(N_CORES)))
    out = np.stack([res.results[c]["y"] for c in range(N_CORES)])
    # y is the step-major spike store: [o2, (i b j)]; output phase i>=WARM2
    # holds spikes for t = j*CHL2 + (i - WARM2), scaled by V.
    out = out.reshape(N_CORES, H2, NSTEP2, B_PER, NCH2).astype(np.float32)
    out = out[:, :, WARM2:]                      # (core, o2, c, b, j)
    out = out.transpose(0, 3, 1, 4, 2)           # core, b, o2, j, c
    out = out.reshape(B_TOT, H2, T) / np.float32(VSP)   # V*s -> s (exact)
    return np.ascontiguousarray(out.astype(np.float32))
